# revision 1
# baseline (speedup 1.0000x reference)
"""Conv2d 3x3 stride1 pad1 (B=32, C_in=128, C_out=256, H=W=56, fp32) on 8 TRN2
NeuronCores, data-parallel over batch (4 images/core), kernels+bias replicated.

Design:
  - Implicit GEMM: contraction dim = C_in = 128 = SBUF partition dim. For each
    (ky,kx) tap, out[co_tile, pix] += w_tap[ci, co_tile].T @ x_shift[ci, pix],
    9 taps accumulated in PSUM (start/stop flags).
  - Zero-padded image strip per batch element in SBUF ([128, 58*58]); the rhs
    of every matmul is a strided [128, 8, 56] slice (8 output rows) whose tap
    shift is just a flat offset ky*58+kx into the strip. N=448 <= one PSUM bank.
  - float32r (TF32) matmuls: 1 cycle/row for N>=256 (4x faster than fp32 path).
    Inputs are pre-rounded to TF32 on the host, which makes every product
    exact in fp32; PSUM accumulates fp32. End-to-end Frobenius rel err vs the
    fp32 reference ~2.8e-4 (pure input-rounding error).
  - Host pre-work: pad + transpose x to [ci, pix] strips, transpose kernels to
    [ci, (tap, co)] so all device DMAs are contiguous; TF32-round both.
  - DMA orchestration: input DMAs chunked (weights tap0 + first 11 rows of the
    first image land in ~1.5us so the PE starts early); input on the SP HWDGE
    ring, output on the Activation ring; bias-add fused into the PSUM->SBUF
    copy (alternating ScalarE activation / VectorE tensor_scalar_add); output
    DMA'd per 8-row group ([128, 448] contiguous).
"""
import sys
import numpy as np

try:
    import concourse.bacc as bacc
except ImportError:
    sys.path.insert(0, '/opt/trn_rl_repo')
    import concourse.bacc as bacc
import concourse.tile as tile
from concourse import mybir
from concourse.bass_utils import run_bass_kernel_spmd

N_CORES = 8
B, B_SH, CI, CO, H, W, K = 32, 4, 128, 256, 56, 56, 3
HP = H + 2
NPIX_PAD = HP * HP
TAPS = [(ky, kx) for ky in range(K) for kx in range(K)]
f32 = mybir.dt.float32
f32r = mybir.dt.float32r
RPT = 8                  # output rows per PSUM tile
N_RG = H // RPT          # 7 row groups
NVAL = RPT * W           # 448


def _tf32_round(a):
    u = np.ascontiguousarray(a, dtype=np.float32).view(np.uint32)
    lsb = (u >> 13) & 1
    u2 = (u + 0xFFF + lsb) & np.uint32(0xFFFFE000)
    return u2.view(np.float32)


def _build_nc(psum_bufs=8, ostage_bufs=6):
    nc = bacc.Bacc("TRN2", target_bir_lowering=False, debug=False)
    xp_d = nc.dram_tensor("xp", [B_SH, CI, NPIX_PAD], f32r, kind="ExternalInput")
    wt_d = nc.dram_tensor("wt", [CI, 9 * CO], f32r, kind="ExternalInput")
    b_d = nc.dram_tensor("bias", [CO], f32, kind="ExternalInput")
    o_d = nc.dram_tensor("out", [B_SH, CO, H, W], f32, kind="ExternalOutput")

    with tile.TileContext(nc) as tc:
        with tc.tile_pool(name="const", bufs=1) as cpool, \
             tc.tile_pool(name="ostage", bufs=ostage_bufs) as opool, \
             tc.tile_pool(name="psum", bufs=psum_bufs, space="PSUM") as ppool:

            xb = [cpool.tile([CI, NPIX_PAD], f32r, name=f"xb{b}")
                  for b in range(B_SH)]
            wr = cpool.tile([CI, 9 * CO], f32r)
            bsb = cpool.tile([128, 2], f32)

            # PE warmup: ~25 dummy matmuls on zeroed operands keep the PE busy
            # through the HAM/p-state ramp (~3.4us at 1.2GHz otherwise) while
            # the input DMAs land; result is never read. Costs ~50ns in the
            # cost-model schedule, saves ~1.5-3us of cold-clock matmuls on HW.
            wt_warm = cpool.tile([128, 64], f32, name="warm")
            nc.gpsimd.memset(wt_warm[:], 0.0)
            wps = ppool.tile([64, 64], f32, tag="ps")
            for _ in range(25):
                nc.tensor.matmul(wps[:], wt_warm[:, :64], wt_warm[:],
                                 start=True, stop=True)

            def dma_x_chunk(b, r):
                if r < N_RG:
                    lo, hi = r * RPT * HP, (r * RPT + RPT) * HP
                else:
                    lo, hi = H * HP, NPIX_PAD
                nc.sync.dma_start(xb[b][:, lo:hi], xp_d.ap()[b][:, lo:hi])

            # first matmul group needs w tap0 + x image0 rows 0..10; the rest
            # of image0 streams before taps 1-8 (PE consumes rows faster than
            # taps early on, and group0 is tap-gated anyway)
            nc.sync.dma_start(wr[:, 0:CO], wt_d.ap()[:, 0:CO])
            dma_x_chunk(0, 0)
            dma_x_chunk(0, 1)
            for r in range(2, N_RG + 1):
                dma_x_chunk(0, r)
            for t in range(1, 9):
                nc.sync.dma_start(wr[:, t * CO:(t + 1) * CO],
                                  wt_d.ap()[:, t * CO:(t + 1) * CO])
            nc.sync.dma_start(bsb[:], b_d.ap().rearrange("(t p) -> p t", p=128))
            for b in range(1, B_SH):
                for r in range(N_RG + 1):
                    dma_x_chunk(b, r)

            n_tile = 0
            for b in range(B_SH):
                xv = xb[b][:].rearrange("p (h w) -> p h w", h=HP)
                for ct in range(2):
                    for rg in range(N_RG):
                        ps = ppool.tile([128, NVAL], f32, tag="ps")
                        for t, (ky, kx) in enumerate(TAPS):
                            rhs = xv[:, rg * RPT + ky: rg * RPT + ky + RPT,
                                     kx:kx + W]
                            off = t * CO + ct * 128
                            nc.tensor.matmul(ps[:], wr[:, off:off + 128], rhs,
                                             start=(t == 0), stop=(t == 8))
                        ot = opool.tile([128, NVAL], f32, tag="ot")
                        if n_tile % 2 == 1:
                            nc.vector.tensor_scalar_add(ot[:], ps[:],
                                                        bsb[:, ct:ct + 1])
                        else:
                            nc.scalar.activation(
                                ot[:], ps[:],
                                mybir.ActivationFunctionType.Identity,
                                bias=bsb[:, ct:ct + 1])
                        nc.scalar.dma_start(
                            o_d.ap()[b, ct * 128:(ct + 1) * 128,
                                     rg * RPT:(rg + 1) * RPT, :]
                            .rearrange("c h w -> c (h w)"), ot[:])
                        n_tile += 1
    nc.compile()
    return nc


def _make_in_maps(x, kernels, bias):
    wt = _tf32_round(np.ascontiguousarray(
        kernels.reshape(CO, CI, 9).transpose(1, 2, 0)).reshape(CI, 9 * CO))
    bias = np.ascontiguousarray(bias, dtype=np.float32)
    in_maps = []
    for c in range(N_CORES):
        xs = x[c * B_SH:(c + 1) * B_SH]
        xp = np.zeros((B_SH, CI, HP, HP), np.float32)
        xp[:, :, 1:H + 1, 1:W + 1] = _tf32_round(xs)
        in_maps.append({"xp": xp.reshape(B_SH, CI, NPIX_PAD),
                        "wt": wt, "bias": bias})
    return in_maps


_NC_CACHE = []


def kernel(x, kernels, bias):
    x = np.ascontiguousarray(np.asarray(x), dtype=np.float32)
    kernels = np.ascontiguousarray(np.asarray(kernels), dtype=np.float32)
    bias = np.ascontiguousarray(np.asarray(bias), dtype=np.float32)
    if not _NC_CACHE:
        _NC_CACHE.append(_build_nc())
    nc = _NC_CACHE[0]
    in_maps = _make_in_maps(x, kernels, bias)
    res = run_bass_kernel_spmd(nc, in_maps, core_ids=list(range(N_CORES)))
    return np.concatenate([r["out"] for r in res.results], axis=0)



# revision 4
# speedup vs baseline: 1.7928x; 1.7928x over previous
"""Conv2d 3x3 stride1 pad1 (B=32, C_in=128, C_out=256, H=W=56, fp32) on 8 TRN2
NeuronCores, data-parallel over batch (4 images/core), kernels+bias replicated.

Design (v2: fp8 DoubleRow):
  - Implicit GEMM with fp8e4 (e4m3) matmuls in MatmulPerfMode.DoubleRow:
    each matmul carries TWO 128-deep k-tiles and costs 0.5 PE cycles per
    output row -- 4x the fp32r rate. The 9-tap x 128-channel contraction is
    done as 9 DoubleRow matmuls per [128co x 448pix] PSUM tile.
  - Precision: an affine two-representation quantization. Each operand a is
    stored as two e4m3 tensors a1 = Q(a), a2 = Q(a - M*(Q(a)-a)) with M=8,
    and the kernel computes psum = a1*b1 + (1/M)*a2*b2 per tap (the two
    k-tiles of one DoubleRow matmul). Up to the exact scalar alpha = M/(M+1)
    applied at drain, this equals (alpha*a1+beta*a2)*(alpha*b1+beta*b2) to
    first order: the leading quantization errors of BOTH operands cancel.
    The 1/M = 1/8 factor is folded as exact power-of-2 scalings into the
    stored second reps (x2*0.5, w2*0.25). Measured end-to-end rel err vs the
    fp32 reference: ~6.6e-3 (gate is 2e-2).
  - Per-batch zero-padded image strip pair in SBUF ([128, 2*58*58] fp8): the
    rhs of every matmul is a 4D strided AP [128ci][2 reps][8 rows][56 cols];
    the tap shift is a flat offset ky*58+kx. Weights [128ci, ct*2304 +
    rep*1152 + tap*128 + co] so each (ct,tap) lhsT is [128][2 reps][128 co].
  - Drain: PSUM -> bf16 staging with fused scale (alpha/(sx*sw)) + bias,
    alternating ScalarE activation / VectorE tensor_scalar; one output DMA
    per (image, co-tile) of [128, 3136] bf16; host upcasts to fp32.
  - PE warmup: small bf16 dummy matmuls keep the Tensor engine busy through
    the p-state ramp while the first input DMAs land.
"""
import sys
import numpy as np
import ml_dtypes

try:
    import concourse.bacc as bacc
except ImportError:
    sys.path.insert(0, '/opt/trn_rl_repo')
    import concourse.bacc as bacc
import concourse.tile as tile
from concourse import mybir
from concourse.ap import AP as APc
from concourse.bass_utils import run_bass_kernel_spmd

N_CORES = 8
B, B_SH, CI, CO, H, W, K = 32, 4, 128, 256, 56, 56, 3
HP = H + 2
NPIX = HP * HP
NPIXP = NPIX + 2         # strip length: +2 so the last garbage tail stays in bounds
TAPS = [(ky, kx) for ky in range(K) for kx in range(K)]
f32 = mybir.dt.float32
bf16 = mybir.dt.bfloat16
f8e4 = mybir.dt.float8e4
E4 = ml_dtypes.float8_e4m3
RPT = 8                  # output rows per PSUM tile
N_RG = H // RPT          # 7 row groups
NVAL = RPT * W           # 448
NOUT = RPT * HP          # 464 matmul out columns (8 rows x 58, incl 16 garbage)

# affine two-rep quantization parameters
SX, SW = 16.0, 64.0      # power-of-2 prescales for x and w
MR = 8.0                 # alpha/beta ratio (power of 2)
CX, DW = 0.5, 0.25       # exact pow2 split of 1/MR across x2 and w2
ALPHA = MR / (MR + 1.0)
GAMMA = float(ALPHA / (SX * SW))   # drain scale
N_WARM = 68
DR = mybir.MatmulPerfMode.DoubleRow
AluOp = mybir.AluOpType


def _build_nc(psum_bufs=8, ostage_bufs=3):
    nc = bacc.Bacc("TRN2", target_bir_lowering=False, debug=False)
    xr_d = nc.dram_tensor("xr", [B_SH, CI, 2 * NPIXP], f8e4, kind="ExternalInput")
    wt_d = nc.dram_tensor("wt", [CI, 2 * 2 * 9 * 128], f8e4, kind="ExternalInput")
    b_d = nc.dram_tensor("bias", [CO], f32, kind="ExternalInput")
    o_d = nc.dram_tensor("out", [B_SH, CO, H, W], bf16, kind="ExternalOutput")

    with tile.TileContext(nc) as tc:
        with tc.tile_pool(name="const", bufs=1) as cpool, \
             tc.tile_pool(name="ostage", bufs=ostage_bufs) as opool, \
             tc.tile_pool(name="psum", bufs=psum_bufs, space="PSUM") as ppool:

            xb = [cpool.tile([CI, 2 * NPIXP], f8e4, name=f"xb{b}")
                  for b in range(B_SH)]
            wr = cpool.tile([CI, 4608], f8e4)
            bsb = cpool.tile([128, 2], f32)

            # PE warmup: small bf16 dummy matmuls on zeroed operands keep the
            # PE busy through the p-state ramp while the first DMAs land.
            warm = cpool.tile([128, 192], bf16, name="warm")
            nc.gpsimd.memset(warm[:], 0.0)
            wps = ppool.tile([128, 64], f32, tag="ps")
            for _ in range(N_WARM):
                nc.tensor.matmul(wps[:], warm[:, 0:128], warm[:, 128:192],
                                 start=True, stop=True)

            # --- input DMAs (all on the SP/sync HWDGE ring) ---
            # image 0 in 4 row chunks (both reps per chunk) so the PE can
            # start early; weights ct-major so ct0's taps land first.
            x0v = xr_d.ap()[0].rearrange("p (r q) -> p r q", r=2)
            xb0v = xb[0][:].rearrange("p (r q) -> p r q", r=2)
            row_chunks = [(0, 870), (870, 1740), (1740, 2610), (2610, NPIXP)]
            lo, hi = row_chunks[0]
            nc.sync.dma_start(xb0v[:, :, lo:hi], x0v[:, :, lo:hi])
            nc.sync.dma_start(wr[:, 0:2304], wt_d.ap()[:, 0:2304])
            nc.sync.dma_start(wr[:, 2304:4608], wt_d.ap()[:, 2304:4608])
            for lo, hi in row_chunks[1:]:
                nc.sync.dma_start(xb0v[:, :, lo:hi], x0v[:, :, lo:hi])
            nc.sync.dma_start(bsb[:], b_d.ap().rearrange("(t p) -> p t", p=128))
            for b in range(1, B_SH):
                nc.sync.dma_start(xb[b][:], xr_d.ap()[b])

            def rhs_ap(b, rg, ky, kx):
                base = xb[b][:]
                off = (rg * RPT + ky) * HP + kx
                return APc(base.tensor, base.offset + off,
                           [[2 * NPIXP, 128], [NPIXP, 2], [1, NOUT]])

            def w_ap(ct, t):
                base = wr[:]
                return APc(base.tensor, base.offset + ct * 2304 + t * 128,
                           [[4608, 128], [1152, 2], [1, 128]])

            n_tile = 0
            for b in range(B_SH):
                for ct in range(2):
                    ot = opool.tile([128, H * W], bf16, tag="ot")
                    for rg in range(N_RG):
                        ps = ppool.tile([128, NOUT], f32, tag="ps")
                        for t, (ky, kx) in enumerate(TAPS):
                            nc.tensor.matmul(ps[:], w_ap(ct, t),
                                             rhs_ap(b, rg, ky, kx),
                                             start=(t == 0), stop=(t == 8),
                                             perf_mode=DR)
                        dst = ot[:, rg * NVAL:(rg + 1) * NVAL] \
                            .rearrange("p (a b) -> p a b", a=RPT)
                        src_v = APc(ps[:].tensor, ps[:].offset,
                                    [[NOUT, 128], [HP, RPT], [1, W]])
                        if n_tile % 2 == 1:
                            nc.vector.tensor_scalar(
                                dst, src_v, GAMMA, bsb[:, ct:ct + 1],
                                op0=AluOp.mult, op1=AluOp.add)
                        else:
                            nc.scalar.activation(
                                dst, src_v,
                                mybir.ActivationFunctionType.Identity,
                                bias=bsb[:, ct:ct + 1], scale=GAMMA)
                        n_tile += 1
                    nc.sync.dma_start(
                        o_d.ap()[b, ct * 128:(ct + 1) * 128]
                        .rearrange("c h w -> c (h w)"), ot[:])
    nc.compile()
    return nc


def _q2(a, scale, c2):
    """Affine two-rep e4m3 quantization: returns (a1, a2_stored)."""
    s = np.asarray(a, np.float32) * scale
    a1 = s.astype(E4)
    v2 = s - np.float32(MR) * (a1.astype(np.float32) - s)
    a2 = v2.astype(E4)
    a2s = (a2.astype(np.float32) * np.float32(c2)).astype(E4)
    return a1, a2s


def _pad(a):
    """[B,CI,H,W] -> zero-padded [B,CI,HP*HP]."""
    p = np.zeros((a.shape[0], CI, HP, HP), E4)
    p[:, :, 1:H + 1, 1:W + 1] = a
    return p.reshape(a.shape[0], CI, NPIX)


def _make_in_maps(x, kernels, bias):
    w1, w2s = _q2(kernels, SW, DW)
    # [rep, CO, CI, 3, 3] -> [ci, ct, rep, tap, co128]
    wq = np.stack([w1, w2s], axis=0).reshape(2, 2, 128, CI, 9)
    wt = np.ascontiguousarray(wq.transpose(3, 1, 0, 4, 2)).reshape(CI, 4608)
    bias = np.ascontiguousarray(bias, dtype=np.float32)
    in_maps = []
    for c in range(N_CORES):
        x1, x2s = _q2(x[c * B_SH:(c + 1) * B_SH], SX, CX)
        xp = np.zeros((B_SH, CI, 2, NPIXP), E4)
        xp[:, :, 0, :NPIX] = _pad(x1)
        xp[:, :, 1, :NPIX] = _pad(x2s)
        in_maps.append({"xr": xp.reshape(B_SH, CI, 2 * NPIXP),
                        "wt": wt, "bias": bias})
    return in_maps


_NC_CACHE = []


def kernel(x, kernels, bias):
    x = np.ascontiguousarray(np.asarray(x), dtype=np.float32)
    kernels = np.ascontiguousarray(np.asarray(kernels), dtype=np.float32)
    bias = np.ascontiguousarray(np.asarray(bias), dtype=np.float32)
    if not _NC_CACHE:
        _NC_CACHE.append(_build_nc())
    nc = _NC_CACHE[0]
    in_maps = _make_in_maps(x, kernels, bias)
    res = run_bass_kernel_spmd(nc, in_maps, core_ids=list(range(N_CORES)))
    return np.concatenate([np.asarray(r["out"], dtype=np.float32)
                           for r in res.results], axis=0)


# revision 10
# speedup vs baseline: 1.8802x; 1.0488x over previous
"""Conv2d 3x3 stride1 pad1 (B=32, C_in=128, C_out=256, H=W=56, fp32) on 8 TRN2
NeuronCores, data-parallel over batch (4 images/core), kernels+bias replicated.

Design (v2: fp8 DoubleRow):
  - Implicit GEMM with fp8e4 (e4m3) matmuls in MatmulPerfMode.DoubleRow:
    each matmul carries TWO 128-deep k-tiles and costs 0.5 PE cycles per
    output row -- 4x the fp32r rate. The 9-tap x 128-channel contraction is
    done as 9 DoubleRow matmuls per [128co x 448pix] PSUM tile.
  - Precision: an affine two-representation quantization. Each operand a is
    stored as two e4m3 tensors a1 = Q(a), a2 = Q(a - M*(Q(a)-a)) with M=8,
    and the kernel computes psum = a1*b1 + (1/M)*a2*b2 per tap (the two
    k-tiles of one DoubleRow matmul). Up to the exact scalar alpha = M/(M+1)
    applied at drain, this equals (alpha*a1+beta*a2)*(alpha*b1+beta*b2) to
    first order: the leading quantization errors of BOTH operands cancel.
    The 1/M = 1/8 factor is folded as exact power-of-2 scalings into the
    stored second reps (x2*0.5, w2*0.25). Measured end-to-end rel err vs the
    fp32 reference: ~6.6e-3 (gate is 2e-2).
  - Per-batch zero-padded image strip pair in SBUF ([128, 2*58*58] fp8): the
    rhs of every matmul is a 4D strided AP [128ci][2 reps][8 rows][56 cols];
    the tap shift is a flat offset ky*58+kx. Weights [128ci, ct*2304 +
    rep*1152 + tap*128 + co] so each (ct,tap) lhsT is [128][2 reps][128 co].
  - Drain: PSUM -> bf16 staging with fused scale (alpha/(sx*sw)) + bias,
    alternating ScalarE activation / VectorE tensor_scalar; one output DMA
    per (image, co-tile) of [128, 3136] bf16; host upcasts to fp32.
  - PE warmup: small bf16 dummy matmuls keep the Tensor engine busy through
    the p-state ramp while the first input DMAs land.
"""
import sys
import numpy as np
import ml_dtypes

try:
    import concourse.bacc as bacc
except ImportError:
    sys.path.insert(0, '/opt/trn_rl_repo')
    import concourse.bacc as bacc
import concourse.tile as tile
from concourse import mybir
from concourse.ap import AP as APc
from concourse.bass_utils import run_bass_kernel_spmd

N_CORES = 8
B, B_SH, CI, CO, H, W, K = 32, 4, 128, 256, 56, 56, 3
HP = H + 2
NPIX = HP * HP
NPIXP = NPIX + 2         # strip length: +2 so the last garbage tail stays in bounds
TAPS = [(ky, kx) for ky in range(K) for kx in range(K)]
f32 = mybir.dt.float32
bf16 = mybir.dt.bfloat16
f8e4 = mybir.dt.float8e4
E4 = ml_dtypes.float8_e4m3
RPT = 8                  # output rows per PSUM tile
N_RG = H // RPT          # 7 row groups
NVAL = RPT * W           # 448
NOUT = RPT * HP          # 464 matmul out columns (8 rows x 58, incl 16 garbage)

# affine two-rep quantization parameters
SX, SW = 16.0, 64.0      # power-of-2 prescales for x and w
MR = 8.0                 # alpha/beta ratio (power of 2)
CX, DW = 0.5, 0.25       # exact pow2 split of 1/MR across x2 and w2
ALPHA = MR / (MR + 1.0)
GAMMA = float(ALPHA / (SX * SW))   # drain scale
N_WARM = 4
DR = mybir.MatmulPerfMode.DoubleRow
AluOp = mybir.AluOpType


def _build_nc(psum_bufs=8, ostage_bufs=3):
    nc = bacc.Bacc("TRN2", target_bir_lowering=False, debug=False)
    xr_d = nc.dram_tensor("xr", [B_SH, CI, 2 * NPIXP], f8e4, kind="ExternalInput")
    wt_d = nc.dram_tensor("wt", [CI, 4616], mybir.dt.uint8, kind="ExternalInput")
    b_d = nc.dram_tensor("bias", [CO], f32, kind="ExternalInput")
    o_d = nc.dram_tensor("out", [B_SH, CO, H, W], bf16, kind="ExternalOutput")

    with tile.TileContext(nc) as tc:
        with tc.tile_pool(name="const", bufs=1) as cpool, \
             tc.tile_pool(name="ostage", bufs=ostage_bufs) as opool, \
             tc.tile_pool(name="psum", bufs=psum_bufs, space="PSUM") as ppool:

            xb = [cpool.tile([CI, 2 * NPIXP], f8e4, name=f"xb{b}")
                  for b in range(B_SH)]
            wr = cpool.tile([CI, 4616], mybir.dt.uint8)
            bsb = wr[:, 4608:4616].bitcast(f32)

            # PE warmup: small bf16 dummy matmuls on zeroed operands keep the
            # PE busy through the p-state ramp while the first DMAs land.
            warm = cpool.tile([128, 192], bf16, name="warm")
            nc.gpsimd.memset(warm[:], 0.0)
            wps = ppool.tile([128, 64], f32, tag="ps")
            for _ in range(N_WARM):
                nc.tensor.matmul(wps[:], warm[:, 0:128], warm[:, 128:192],
                                 start=True, stop=True)

            # --- input DMAs (all on the SP/sync HWDGE ring) ---
            # image 0 in 4 row chunks (both reps per chunk) so the PE can
            # start early; weights ct-major so ct0's taps land first.
            x0v = xr_d.ap()[0].rearrange("p (r q) -> p r q", r=2)
            xb0v = xb[0][:].rearrange("p (r q) -> p r q", r=2)
            row_chunks = [(0, 638), (638, 1740), (1740, 2610), (2610, NPIXP)]
            nc.sync.dma_start(wr[:, 0:2304], wt_d.ap()[:, 0:2304])
            lo, hi = row_chunks[0]
            nc.sync.dma_start(xb0v[:, :, lo:hi], x0v[:, :, lo:hi])
            lo, hi = row_chunks[1]
            nc.sync.dma_start(xb0v[:, :, lo:hi], x0v[:, :, lo:hi])
            nc.sync.dma_start(wr[:, 2304:4616], wt_d.ap()[:, 2304:4616])
            for lo, hi in row_chunks[2:]:
                nc.sync.dma_start(xb0v[:, :, lo:hi], x0v[:, :, lo:hi])
            for b in range(1, B_SH):
                nc.sync.dma_start(xb[b][:], xr_d.ap()[b])

            def rhs_ap(b, rg, ky, kx):
                base = xb[b][:]
                off = (rg * RPT + ky) * HP + kx
                return APc(base.tensor, base.offset + off,
                           [[2 * NPIXP, 128], [NPIXP, 2], [1, NOUT]])

            def w_ap(ct, t):
                base = wr[:]
                return APc(base.tensor, base.offset + ct * 2304 + t * 128,
                           [[4616, 128], [1152, 2], [1, 128]]).bitcast(f8e4)

            n_tile = 0
            for b in range(B_SH):
                for ct in range(2):
                    last_tile = (b == B_SH - 1 and ct == 1)
                    # row groups: normally 7x8; the final tile ends with a
                    # 7-row and a 1-row group so the tail drain+DMA is tiny
                    groups = ([(r * RPT, RPT) for r in range(N_RG)]
                              if not last_tile else
                              [(r * RPT, RPT) for r in range(6)] + [(48, 7), (55, 1)])
                    ot = opool.tile([128, H * W], bf16, tag="ot")
                    for gi, (r0, nr) in enumerate(groups):
                        nout = (nr - 1) * HP + W + (K - 1)
                        ps = ppool.tile([128, nout], f32, tag="ps")
                        for t, (ky, kx) in enumerate(TAPS):
                            base = xb[b][:]
                            off = (r0 + ky) * HP + kx
                            rhs = APc(base.tensor, base.offset + off,
                                      [[2 * NPIXP, 128], [NPIXP, 2], [1, nout]])
                            nc.tensor.matmul(ps[:], w_ap(ct, t), rhs,
                                             start=(t == 0), stop=(t == 8),
                                             perf_mode=DR)
                        dst = ot[:, r0 * W:(r0 + nr) * W] \
                            .rearrange("p (a b) -> p a b", a=nr)
                        src_v = APc(ps[:].tensor, ps[:].offset,
                                    [[nout, 128], [HP, nr], [1, W]])
                        if n_tile % 2 == 1:
                            nc.vector.tensor_scalar(
                                dst, src_v, GAMMA, bsb[:, ct:ct + 1],
                                op0=AluOp.mult, op1=AluOp.add)
                        else:
                            nc.scalar.activation(
                                dst, src_v,
                                mybir.ActivationFunctionType.Identity,
                                bias=bsb[:, ct:ct + 1], scale=GAMMA)
                        if last_tile and r0 + nr <= 48:
                            # stream the final co-tile per row group; the two
                            # trailing groups (rows 48-55) go as one small DMA
                            nc.sync.dma_start(
                                o_d.ap()[b, 128:256, r0:r0 + nr]
                                .rearrange("c h w -> c (h w)"),
                                ot[:, r0 * W:(r0 + nr) * W])
                        elif last_tile and r0 + nr == H:
                            nc.sync.dma_start(
                                o_d.ap()[b, 128:256, 48:H]
                                .rearrange("c h w -> c (h w)"),
                                ot[:, 48 * W:H * W])
                        n_tile += 1
                    if not last_tile:
                        nc.sync.dma_start(
                            o_d.ap()[b, ct * 128:(ct + 1) * 128]
                            .rearrange("c h w -> c (h w)"), ot[:])
    nc.compile()
    return nc


def _q2(a, scale, c2):
    """Affine two-rep e4m3 quantization: returns (a1, a2_stored)."""
    s = np.asarray(a, np.float32) * scale
    a1 = s.astype(E4)
    v2 = s - np.float32(MR) * (a1.astype(np.float32) - s)
    a2 = v2.astype(E4)
    a2s = (a2.astype(np.float32) * np.float32(c2)).astype(E4)
    return a1, a2s


def _pad(a):
    """[B,CI,H,W] -> zero-padded [B,CI,HP*HP]."""
    p = np.zeros((a.shape[0], CI, HP, HP), E4)
    p[:, :, 1:H + 1, 1:W + 1] = a
    return p.reshape(a.shape[0], CI, NPIX)


def _make_in_maps(x, kernels, bias):
    w1, w2s = _q2(kernels, SW, DW)
    # [rep, CO, CI, 3, 3] -> [ci, ct, rep, tap, co128]
    wq = np.stack([w1, w2s], axis=0).reshape(2, 2, 128, CI, 9)
    wt = np.zeros((CI, 4616), np.uint8)
    wt[:, :4608] = np.ascontiguousarray(wq.transpose(3, 1, 0, 4, 2)).reshape(CI, 4608).view(np.uint8)
    bias = np.ascontiguousarray(bias, dtype=np.float32)
    # bias packed as raw fp32 bytes into the last 8 wt columns:
    # row p holds [bias[p], bias[128+p]] so wr[:,4608:4616].bitcast(f32) is [128,2]
    bpack = np.stack([bias[:128], bias[128:]], axis=1)  # [128, 2] f32
    wt[:, 4608:4616] = bpack.view(np.uint8).reshape(128, 8)
    in_maps = []
    for c in range(N_CORES):
        x1, x2s = _q2(x[c * B_SH:(c + 1) * B_SH], SX, CX)
        xp = np.zeros((B_SH, CI, 2, NPIXP), E4)
        xp[:, :, 0, :NPIX] = _pad(x1)
        xp[:, :, 1, :NPIX] = _pad(x2s)
        in_maps.append({"xr": xp.reshape(B_SH, CI, 2 * NPIXP),
                        "wt": wt, "bias": bias})
    return in_maps


_NC_CACHE = []


def kernel(x, kernels, bias):
    x = np.ascontiguousarray(np.asarray(x), dtype=np.float32)
    kernels = np.ascontiguousarray(np.asarray(kernels), dtype=np.float32)
    bias = np.ascontiguousarray(np.asarray(bias), dtype=np.float32)
    if not _NC_CACHE:
        _NC_CACHE.append(_build_nc())
    nc = _NC_CACHE[0]
    in_maps = _make_in_maps(x, kernels, bias)
    res = run_bass_kernel_spmd(nc, in_maps, core_ids=list(range(N_CORES)))
    return np.concatenate([np.asarray(r["out"], dtype=np.float32)
                           for r in res.results], axis=0)


# revision 13
# speedup vs baseline: 2.0758x; 1.1040x over previous
"""Conv2d 3x3 stride1 pad1 (B=32, C_in=128, C_out=256, H=W=56, fp32) on 8 TRN2
NeuronCores, data-parallel over batch (4 images/core), kernels+bias replicated.

Design (v2: fp8 DoubleRow):
  - Implicit GEMM with fp8e4 (e4m3) matmuls in MatmulPerfMode.DoubleRow:
    each matmul carries TWO 128-deep k-tiles and costs 0.5 PE cycles per
    output row -- 4x the fp32r rate. The 9-tap x 128-channel contraction is
    done as 9 DoubleRow matmuls per [128co x 448pix] PSUM tile.
  - Precision: an affine two-representation quantization. Each operand a is
    stored as two e4m3 tensors a1 = Q(a), a2 = Q(a - M*(Q(a)-a)) with M=8,
    and the kernel computes psum = a1*b1 + (1/M)*a2*b2 per tap (the two
    k-tiles of one DoubleRow matmul). Up to the exact scalar alpha = M/(M+1)
    applied at drain, this equals (alpha*a1+beta*a2)*(alpha*b1+beta*b2) to
    first order: the leading quantization errors of BOTH operands cancel.
    The 1/M = 1/8 factor is folded as exact power-of-2 scalings into the
    stored second reps (x2*0.5, w2*0.25). Measured end-to-end rel err vs the
    fp32 reference: ~6.6e-3 (gate is 2e-2).
  - Per-batch zero-padded image strip pair in SBUF ([128, 2*58*58] fp8): the
    rhs of every matmul is a 4D strided AP [128ci][2 reps][8 rows][56 cols];
    the tap shift is a flat offset ky*58+kx. Weights [128ci, ct*2304 +
    rep*1152 + tap*128 + co] so each (ct,tap) lhsT is [128][2 reps][128 co].
  - Drain: PSUM -> bf16 staging with fused scale (alpha/(sx*sw)) + bias,
    alternating ScalarE activation / VectorE tensor_scalar; one output DMA
    per (image, co-tile) of [128, 3136] bf16; host upcasts to fp32.
  - PE warmup: small bf16 dummy matmuls keep the Tensor engine busy through
    the p-state ramp while the first input DMAs land.
"""
import sys
import numpy as np
import ml_dtypes

try:
    import concourse.bacc as bacc
except ImportError:
    sys.path.insert(0, '/opt/trn_rl_repo')
    import concourse.bacc as bacc
import concourse.tile as tile
from concourse import mybir
from concourse.ap import AP as APc
from concourse.bass_utils import run_bass_kernel_spmd

N_CORES = 8
B, B_SH, CI, CO, H, W, K = 32, 4, 128, 256, 56, 56, 3
HP = H + 2
NPIX = HP * HP
NPIXP = NPIX + 2         # strip length: +2 so the last garbage tail stays in bounds
TAPS = [(ky, kx) for ky in range(K) for kx in range(K)]
f32 = mybir.dt.float32
bf16 = mybir.dt.bfloat16
f8e4 = mybir.dt.float8e4
E4 = ml_dtypes.float8_e4m3
RPT = 8                  # output rows per PSUM tile
N_RG = H // RPT          # 7 row groups
NVAL = RPT * W           # 448
NOUT = RPT * HP          # 464 matmul out columns (8 rows x 58, incl 16 garbage)

# affine two-rep quantization parameters
SX, SW = 16.0, 64.0      # power-of-2 prescales for x and w
MR = 8.0                 # alpha/beta ratio (power of 2)
CX, DW = 0.5, 0.25       # exact pow2 split of 1/MR across x2 and w2
ALPHA = MR / (MR + 1.0)
GAMMA = float(ALPHA / (SX * SW))   # drain scale
N_WARM = 4
DR = mybir.MatmulPerfMode.DoubleRow
AluOp = mybir.AluOpType


def _build_nc(psum_bufs=8, ostage_bufs=3):
    nc = bacc.Bacc("TRN2", target_bir_lowering=False, debug=False)
    xr_d = nc.dram_tensor("xr", [B_SH, CI, 2 * NPIXP], f8e4, kind="ExternalInput")
    wt_d = nc.dram_tensor("wt", [CI, 4616], mybir.dt.uint8, kind="ExternalInput")
    b_d = nc.dram_tensor("bias", [CO], f32, kind="ExternalInput")
    o_d = nc.dram_tensor("out", [B_SH, CO, H, W], bf16, kind="ExternalOutput")

    with tile.TileContext(nc) as tc:
        with tc.tile_pool(name="const", bufs=1) as cpool, \
             tc.tile_pool(name="ostage", bufs=ostage_bufs) as opool, \
             tc.tile_pool(name="psum", bufs=psum_bufs, space="PSUM") as ppool:

            xb = [cpool.tile([CI, 2 * NPIXP], f8e4, name=f"xb{b}")
                  for b in range(B_SH)]
            wr = cpool.tile([CI, 4616], mybir.dt.uint8)
            bsb = wr[:, 4608:4616].bitcast(f32)

            # PE warmup: small bf16 dummy matmuls on zeroed operands keep the
            # PE busy through the p-state ramp while the first DMAs land.
            warm = cpool.tile([128, 192], bf16, name="warm")
            nc.gpsimd.memset(warm[:], 0.0)
            wps = ppool.tile([128, 64], f32, tag="ps")
            for _ in range(N_WARM):
                nc.tensor.matmul(wps[:], warm[:, 0:128], warm[:, 128:192],
                                 start=True, stop=True)

            # --- input DMAs (all on the SP/sync HWDGE ring) ---
            # image 0 in 4 row chunks (both reps per chunk) so the PE can
            # start early; weights ct-major so ct0's taps land first.
            x0v = xr_d.ap()[0].rearrange("p (r q) -> p r q", r=2)
            xb0v = xb[0][:].rearrange("p (r q) -> p r q", r=2)
            row_chunks = [(0, 638), (638, 1740), (1740, 2610), (2610, NPIXP)]
            nc.sync.dma_start(wr[:, 0:2304], wt_d.ap()[:, 0:2304])
            lo, hi = row_chunks[0]
            nc.sync.dma_start(xb0v[:, :, lo:hi], x0v[:, :, lo:hi])
            lo, hi = row_chunks[1]
            nc.sync.dma_start(xb0v[:, :, lo:hi], x0v[:, :, lo:hi])
            nc.sync.dma_start(wr[:, 2304:4616], wt_d.ap()[:, 2304:4616])
            for lo, hi in row_chunks[2:]:
                nc.sync.dma_start(xb0v[:, :, lo:hi], x0v[:, :, lo:hi])
            for b in range(1, B_SH):
                nc.sync.dma_start(xb[b][:], xr_d.ap()[b])

            def rhs_ap(b, rg, ky, kx):
                base = xb[b][:]
                off = (rg * RPT + ky) * HP + kx
                return APc(base.tensor, base.offset + off,
                           [[2 * NPIXP, 128], [NPIXP, 2], [1, NOUT]])

            def w_ap(ct, t):
                base = wr[:]
                return APc(base.tensor, base.offset + ct * 2304 + t * 128,
                           [[4616, 128], [1152, 2], [1, 128]]).bitcast(f8e4)

            n_tile = 0
            for b in range(B_SH):
                for ct in range(2):
                    last_tile = (b == B_SH - 1 and ct == 1)
                    # row groups: normally 7x8; the final tile ends with a
                    # 7-row and a 1-row group so the tail drain+DMA is tiny
                    groups = ([(r * RPT, RPT) for r in range(N_RG)]
                              if not last_tile else
                              [(r * RPT, RPT) for r in range(6)] + [(48, 7), (55, 1)])
                    ot = opool.tile([128, H * W], bf16, tag="ot")
                    for gi, (r0, nr) in enumerate(groups):
                        nout = (nr - 1) * HP + W + (K - 1)
                        ps = ppool.tile([128, nout], f32, tag="ps")
                        base = xb[b][:]
                        # taps 0 and 8 ride one DoubleRow matmul as two
                        # single-rep k-tiles (weights pre-scaled by 1/alpha);
                        # taps 1..7 are affine pairs (rep1 + rep2 k-tiles)
                        rhs_s = APc(base.tensor, base.offset + r0 * HP,
                                    [[2 * NPIXP, 128], [2 * HP + 2, 2],
                                     [1, nout]])
                        lhs_s = APc(wr[:].tensor, wr[:].offset + ct * 2304,
                                    [[4616, 128], [1024, 2], [1, 128]]) \
                            .bitcast(f8e4)
                        nc.tensor.matmul(ps[:], lhs_s, rhs_s,
                                         start=True, stop=False, perf_mode=DR)
                        for t in range(1, 8):
                            ky, kx = TAPS[t]
                            off = (r0 + ky) * HP + kx
                            rhs = APc(base.tensor, base.offset + off,
                                      [[2 * NPIXP, 128], [NPIXP, 2], [1, nout]])
                            nc.tensor.matmul(ps[:], w_ap(ct, t), rhs,
                                             start=False, stop=(t == 7),
                                             perf_mode=DR)
                        dst = ot[:, r0 * W:(r0 + nr) * W] \
                            .rearrange("p (a b) -> p a b", a=nr)
                        src_v = APc(ps[:].tensor, ps[:].offset,
                                    [[nout, 128], [HP, nr], [1, W]])
                        if n_tile % 2 == 1:
                            nc.vector.tensor_scalar(
                                dst, src_v, GAMMA, bsb[:, ct:ct + 1],
                                op0=AluOp.mult, op1=AluOp.add)
                        else:
                            nc.scalar.activation(
                                dst, src_v,
                                mybir.ActivationFunctionType.Identity,
                                bias=bsb[:, ct:ct + 1], scale=GAMMA)
                        if last_tile and r0 + nr <= 48:
                            # stream the final co-tile per row group; the two
                            # trailing groups (rows 48-55) go as one small DMA
                            nc.sync.dma_start(
                                o_d.ap()[b, 128:256, r0:r0 + nr]
                                .rearrange("c h w -> c (h w)"),
                                ot[:, r0 * W:(r0 + nr) * W])
                        elif last_tile and r0 + nr == H:
                            nc.sync.dma_start(
                                o_d.ap()[b, 128:256, 48:H]
                                .rearrange("c h w -> c (h w)"),
                                ot[:, 48 * W:H * W])
                        n_tile += 1
                    if not last_tile:
                        nc.sync.dma_start(
                            o_d.ap()[b, ct * 128:(ct + 1) * 128]
                            .rearrange("c h w -> c (h w)"), ot[:])
    nc.compile()
    return nc


def _q2(a, scale, c2):
    """Affine two-rep e4m3 quantization: returns (a1, a2_stored)."""
    s = np.asarray(a, np.float32) * scale
    a1 = s.astype(E4)
    v2 = s - np.float32(MR) * (a1.astype(np.float32) - s)
    a2 = v2.astype(E4)
    a2s = (a2.astype(np.float32) * np.float32(c2)).astype(E4)
    return a1, a2s


def _pad(a):
    """[B,CI,H,W] -> zero-padded [B,CI,HP*HP]."""
    p = np.zeros((a.shape[0], CI, HP, HP), E4)
    p[:, :, 1:H + 1, 1:W + 1] = a
    return p.reshape(a.shape[0], CI, NPIX)


def _make_in_maps(x, kernels, bias):
    ksc = np.asarray(kernels, np.float32).copy()
    # taps (0,0) and (2,2) are single-rep: pre-divide by ALPHA so the uniform
    # alpha drain scale nets to 1 for them (exact algebra, same RTN error)
    ksc[:, :, 0, 0] *= np.float32((MR + 1.0) / MR)
    ksc[:, :, 2, 2] *= np.float32((MR + 1.0) / MR)
    w1, w2s = _q2(ksc, SW, DW)
    w2s[:, :, 0, 0] = 0
    w2s[:, :, 2, 2] = 0
    # [rep, CO, CI, 3, 3] -> [ci, ct, rep, tap, co128]
    wq = np.stack([w1, w2s], axis=0).reshape(2, 2, 128, CI, 9)
    wt = np.zeros((CI, 4616), np.uint8)
    wt[:, :4608] = np.ascontiguousarray(wq.transpose(3, 1, 0, 4, 2)).reshape(CI, 4608).view(np.uint8)
    bias = np.ascontiguousarray(bias, dtype=np.float32)
    # bias packed as raw fp32 bytes into the last 8 wt columns:
    # row p holds [bias[p], bias[128+p]] so wr[:,4608:4616].bitcast(f32) is [128,2]
    bpack = np.stack([bias[:128], bias[128:]], axis=1)  # [128, 2] f32
    wt[:, 4608:4616] = bpack.view(np.uint8).reshape(128, 8)
    in_maps = []
    for c in range(N_CORES):
        x1, x2s = _q2(x[c * B_SH:(c + 1) * B_SH], SX, CX)
        xp = np.zeros((B_SH, CI, 2, NPIXP), E4)
        xp[:, :, 0, :NPIX] = _pad(x1)
        xp[:, :, 1, :NPIX] = _pad(x2s)
        in_maps.append({"xr": xp.reshape(B_SH, CI, 2 * NPIXP),
                        "wt": wt, "bias": bias})
    return in_maps


_NC_CACHE = []


def kernel(x, kernels, bias):
    x = np.ascontiguousarray(np.asarray(x), dtype=np.float32)
    kernels = np.ascontiguousarray(np.asarray(kernels), dtype=np.float32)
    bias = np.ascontiguousarray(np.asarray(bias), dtype=np.float32)
    if not _NC_CACHE:
        _NC_CACHE.append(_build_nc())
    nc = _NC_CACHE[0]
    in_maps = _make_in_maps(x, kernels, bias)
    res = run_bass_kernel_spmd(nc, in_maps, core_ids=list(range(N_CORES)))
    return np.concatenate([np.asarray(r["out"], dtype=np.float32)
                           for r in res.results], axis=0)


# revision 15
# speedup vs baseline: 2.0784x; 1.0013x over previous
"""Conv2d 3x3 stride1 pad1 (B=32, C_in=128, C_out=256, H=W=56, fp32) on 8 TRN2
NeuronCores, data-parallel over batch (4 images/core), kernels+bias replicated.

Design (v2: fp8 DoubleRow):
  - Implicit GEMM with fp8e4 (e4m3) matmuls in MatmulPerfMode.DoubleRow:
    each matmul carries TWO 128-deep k-tiles and costs 0.5 PE cycles per
    output row -- 4x the fp32r rate. The 9-tap x 128-channel contraction is
    done as 9 DoubleRow matmuls per [128co x 448pix] PSUM tile.
  - Precision: an affine two-representation quantization. Each operand a is
    stored as two e4m3 tensors a1 = Q(a), a2 = Q(a - M*(Q(a)-a)) with M=8,
    and the kernel computes psum = a1*b1 + (1/M)*a2*b2 per tap (the two
    k-tiles of one DoubleRow matmul). Up to the exact scalar alpha = M/(M+1)
    applied at drain, this equals (alpha*a1+beta*a2)*(alpha*b1+beta*b2) to
    first order: the leading quantization errors of BOTH operands cancel.
    The 1/M = 1/8 factor is folded as exact power-of-2 scalings into the
    stored second reps (x2*0.5, w2*0.25). Measured end-to-end rel err vs the
    fp32 reference: ~6.6e-3 (gate is 2e-2).
  - Per-batch zero-padded image strip pair in SBUF ([128, 2*58*58] fp8): the
    rhs of every matmul is a 4D strided AP [128ci][2 reps][8 rows][56 cols];
    the tap shift is a flat offset ky*58+kx. Weights [128ci, ct*2304 +
    rep*1152 + tap*128 + co] so each (ct,tap) lhsT is [128][2 reps][128 co].
  - Drain: PSUM -> bf16 staging with fused scale (alpha/(sx*sw)) + bias,
    alternating ScalarE activation / VectorE tensor_scalar; one output DMA
    per (image, co-tile) of [128, 3136] bf16; host upcasts to fp32.
  - PE warmup: small bf16 dummy matmuls keep the Tensor engine busy through
    the p-state ramp while the first input DMAs land.
"""
import sys
import numpy as np
import ml_dtypes

try:
    import concourse.bacc as bacc
except ImportError:
    sys.path.insert(0, '/opt/trn_rl_repo')
    import concourse.bacc as bacc
import concourse.tile as tile
from concourse import mybir
from concourse.ap import AP as APc
from concourse.bass_utils import run_bass_kernel_spmd

N_CORES = 8
B, B_SH, CI, CO, H, W, K = 32, 4, 128, 256, 56, 56, 3
HP = H + 2
NPIX = HP * HP
NPIXP = NPIX + 2         # strip length: +2 so the last garbage tail stays in bounds
TAPS = [(ky, kx) for ky in range(K) for kx in range(K)]
f32 = mybir.dt.float32
bf16 = mybir.dt.bfloat16
f8e4 = mybir.dt.float8e4
E4 = ml_dtypes.float8_e4m3
RPT = 8                  # output rows per PSUM tile
N_RG = H // RPT          # 7 row groups
NVAL = RPT * W           # 448
NOUT = RPT * HP          # 464 matmul out columns (8 rows x 58, incl 16 garbage)

# affine two-rep quantization parameters
SX, SW = 16.0, 64.0      # power-of-2 prescales for x and w
MR = 8.0                 # alpha/beta ratio (power of 2)
CX, DW = 0.5, 0.25       # exact pow2 split of 1/MR across x2 and w2
ALPHA = MR / (MR + 1.0)
GAMMA = float(ALPHA / (SX * SW))   # drain scale
N_WARM = 4
DR = mybir.MatmulPerfMode.DoubleRow
AluOp = mybir.AluOpType


def _build_nc(psum_bufs=8, ostage_bufs=3):
    nc = bacc.Bacc("TRN2", target_bir_lowering=False, debug=False)
    xr_d = nc.dram_tensor("xr", [B_SH, CI, 2 * NPIXP], f8e4, kind="ExternalInput")
    wt_d = nc.dram_tensor("wt", [CI, 4616], mybir.dt.uint8, kind="ExternalInput")
    b_d = nc.dram_tensor("bias", [CO], f32, kind="ExternalInput")
    o_d = nc.dram_tensor("out", [B_SH, CO, H, W], bf16, kind="ExternalOutput")

    with tile.TileContext(nc) as tc:
        with tc.tile_pool(name="const", bufs=1) as cpool, \
             tc.tile_pool(name="ostage", bufs=ostage_bufs) as opool, \
             tc.tile_pool(name="psum", bufs=psum_bufs, space="PSUM") as ppool:

            xb = [cpool.tile([CI, 2 * NPIXP], f8e4, name=f"xb{b}")
                  for b in range(B_SH)]
            wr = cpool.tile([CI, 4616], mybir.dt.uint8)
            bsb = wr[:, 4608:4616].bitcast(f32)

            # PE warmup: small bf16 dummy matmuls on zeroed operands keep the
            # PE busy through the p-state ramp while the first DMAs land.
            warm = cpool.tile([128, 192], bf16, name="warm")
            nc.vector.memset(warm[:], 0.0)
            wps = ppool.tile([128, 64], f32, tag="ps")
            for _ in range(N_WARM):
                nc.tensor.matmul(wps[:], warm[:, 0:128], warm[:, 128:192],
                                 start=True, stop=True)

            # --- input DMAs (all on the SP/sync HWDGE ring) ---
            # image 0 in 4 row chunks (both reps per chunk) so the PE can
            # start early; weights ct-major so ct0's taps land first.
            x0v = xr_d.ap()[0].rearrange("p (r q) -> p r q", r=2)
            xb0v = xb[0][:].rearrange("p (r q) -> p r q", r=2)
            row_chunks = [(0, 582), (582, 1740), (1740, 2610), (2610, NPIXP)]
            nc.sync.dma_start(wr[:, 0:2304], wt_d.ap()[:, 0:2304])
            lo, hi = row_chunks[0]
            nc.sync.dma_start(xb0v[:, :, lo:hi], x0v[:, :, lo:hi])
            lo, hi = row_chunks[1]
            nc.sync.dma_start(xb0v[:, :, lo:hi], x0v[:, :, lo:hi])
            nc.sync.dma_start(wr[:, 2304:4616], wt_d.ap()[:, 2304:4616])
            for lo, hi in row_chunks[2:]:
                nc.sync.dma_start(xb0v[:, :, lo:hi], x0v[:, :, lo:hi])
            for b in range(1, B_SH):
                nc.sync.dma_start(xb[b][:], xr_d.ap()[b])

            def rhs_ap(b, rg, ky, kx):
                base = xb[b][:]
                off = (rg * RPT + ky) * HP + kx
                return APc(base.tensor, base.offset + off,
                           [[2 * NPIXP, 128], [NPIXP, 2], [1, NOUT]])

            def w_ap(ct, t):
                base = wr[:]
                return APc(base.tensor, base.offset + ct * 2304 + t * 128,
                           [[4616, 128], [1152, 2], [1, 128]]).bitcast(f8e4)

            n_tile = 0
            for b in range(B_SH):
                for ct in range(2):
                    last_tile = (b == B_SH - 1 and ct == 1)
                    # row groups: normally 7x8; the final tile ends with a
                    # 7-row and a 1-row group so the tail drain+DMA is tiny
                    groups = ([(r * RPT, RPT) for r in range(N_RG)]
                              if not last_tile else
                              [(r * RPT, RPT) for r in range(6)] + [(48, 7), (55, 1)])
                    ot = opool.tile([128, H * W], bf16, tag="ot")
                    for gi, (r0, nr) in enumerate(groups):
                        nout = (nr - 1) * HP + W + (K - 1)
                        ps = ppool.tile([128, nout], f32, tag="ps")
                        base = xb[b][:]
                        # taps 0 and 8 ride one DoubleRow matmul as two
                        # single-rep k-tiles (weights pre-scaled by 1/alpha);
                        # taps 1..7 are affine pairs (rep1 + rep2 k-tiles)
                        rhs_s = APc(base.tensor, base.offset + r0 * HP,
                                    [[2 * NPIXP, 128], [2 * HP + 2, 2],
                                     [1, nout]])
                        lhs_s = APc(wr[:].tensor, wr[:].offset + ct * 2304,
                                    [[4616, 128], [1024, 2], [1, 128]]) \
                            .bitcast(f8e4)
                        nc.tensor.matmul(ps[:], lhs_s, rhs_s,
                                         start=True, stop=False, perf_mode=DR)
                        for t in range(1, 8):
                            ky, kx = TAPS[t]
                            off = (r0 + ky) * HP + kx
                            rhs = APc(base.tensor, base.offset + off,
                                      [[2 * NPIXP, 128], [NPIXP, 2], [1, nout]])
                            nc.tensor.matmul(ps[:], w_ap(ct, t), rhs,
                                             start=False, stop=(t == 7),
                                             perf_mode=DR)
                        dst = ot[:, r0 * W:(r0 + nr) * W] \
                            .rearrange("p (a b) -> p a b", a=nr)
                        src_v = APc(ps[:].tensor, ps[:].offset,
                                    [[nout, 128], [HP, nr], [1, W]])
                        if (n_tile + last_tile) % 2 == 1:
                            nc.vector.tensor_scalar(
                                dst, src_v, GAMMA, bsb[:, ct:ct + 1],
                                op0=AluOp.mult, op1=AluOp.add)
                        else:
                            nc.scalar.activation(
                                dst, src_v,
                                mybir.ActivationFunctionType.Identity,
                                bias=bsb[:, ct:ct + 1], scale=GAMMA)
                        if last_tile and r0 + nr <= 48:
                            # stream the final co-tile per row group; the two
                            # trailing groups (rows 48-55) go as one small DMA
                            nc.sync.dma_start(
                                o_d.ap()[b, 128:256, r0:r0 + nr]
                                .rearrange("c h w -> c (h w)"),
                                ot[:, r0 * W:(r0 + nr) * W])
                        elif last_tile and r0 + nr == H:
                            nc.sync.dma_start(
                                o_d.ap()[b, 128:256, 48:H]
                                .rearrange("c h w -> c (h w)"),
                                ot[:, 48 * W:H * W])
                        n_tile += 1
                    if not last_tile:
                        nc.sync.dma_start(
                            o_d.ap()[b, ct * 128:(ct + 1) * 128]
                            .rearrange("c h w -> c (h w)"), ot[:])
    nc.compile()
    return nc


def _q2(a, scale, c2):
    """Affine two-rep e4m3 quantization: returns (a1, a2_stored)."""
    s = np.asarray(a, np.float32) * scale
    a1 = s.astype(E4)
    v2 = s - np.float32(MR) * (a1.astype(np.float32) - s)
    a2 = v2.astype(E4)
    a2s = (a2.astype(np.float32) * np.float32(c2)).astype(E4)
    return a1, a2s


def _pad(a):
    """[B,CI,H,W] -> zero-padded [B,CI,HP*HP]."""
    p = np.zeros((a.shape[0], CI, HP, HP), E4)
    p[:, :, 1:H + 1, 1:W + 1] = a
    return p.reshape(a.shape[0], CI, NPIX)


def _make_in_maps(x, kernels, bias):
    ksc = np.asarray(kernels, np.float32).copy()
    # taps (0,0) and (2,2) are single-rep: pre-divide by ALPHA so the uniform
    # alpha drain scale nets to 1 for them (exact algebra, same RTN error)
    ksc[:, :, 0, 0] *= np.float32((MR + 1.0) / MR)
    ksc[:, :, 2, 2] *= np.float32((MR + 1.0) / MR)
    w1, w2s = _q2(ksc, SW, DW)
    w2s[:, :, 0, 0] = 0
    w2s[:, :, 2, 2] = 0
    # [rep, CO, CI, 3, 3] -> [ci, ct, rep, tap, co128]
    wq = np.stack([w1, w2s], axis=0).reshape(2, 2, 128, CI, 9)
    wt = np.zeros((CI, 4616), np.uint8)
    wt[:, :4608] = np.ascontiguousarray(wq.transpose(3, 1, 0, 4, 2)).reshape(CI, 4608).view(np.uint8)
    bias = np.ascontiguousarray(bias, dtype=np.float32)
    # bias packed as raw fp32 bytes into the last 8 wt columns:
    # row p holds [bias[p], bias[128+p]] so wr[:,4608:4616].bitcast(f32) is [128,2]
    bpack = np.stack([bias[:128], bias[128:]], axis=1)  # [128, 2] f32
    wt[:, 4608:4616] = bpack.view(np.uint8).reshape(128, 8)
    in_maps = []
    for c in range(N_CORES):
        x1, x2s = _q2(x[c * B_SH:(c + 1) * B_SH], SX, CX)
        xp = np.zeros((B_SH, CI, 2, NPIXP), E4)
        xp[:, :, 0, :NPIX] = _pad(x1)
        xp[:, :, 1, :NPIX] = _pad(x2s)
        in_maps.append({"xr": xp.reshape(B_SH, CI, 2 * NPIXP),
                        "wt": wt, "bias": bias})
    return in_maps


_NC_CACHE = []


def kernel(x, kernels, bias):
    x = np.ascontiguousarray(np.asarray(x), dtype=np.float32)
    kernels = np.ascontiguousarray(np.asarray(kernels), dtype=np.float32)
    bias = np.ascontiguousarray(np.asarray(bias), dtype=np.float32)
    if not _NC_CACHE:
        _NC_CACHE.append(_build_nc())
    nc = _NC_CACHE[0]
    in_maps = _make_in_maps(x, kernels, bias)
    res = run_bass_kernel_spmd(nc, in_maps, core_ids=list(range(N_CORES)))
    return np.concatenate([np.asarray(r["out"], dtype=np.float32)
                           for r in res.results], axis=0)
